# revision 11
# baseline (speedup 1.0000x reference)
"""Low-rank cross-attention on 8 Trainium2 NeuronCores (Bass/Tile).

Problem: out = (softmax((tgt@Wq.T)(memory@Wk.T).T / sqrt(r)) @ (memory@Wv.T)) @ Wo.T
Shapes: tgt/memory [4, 2048, 1024], r=128, d_model=1024.

Sharding: core c in 0..7 handles batch b=c//2 and query-half h=c%2
(1024 query tokens) against the full 2048-token memory of its batch.
No collectives.

Key layout trick: all DRAM inputs are host-pre-transposed so every
on-device matmul has its contraction dim on the SBUF partition axis:
  qT [r,T]   = WqT.T @ tgtT          (contract d)
  kT [r,S]   = WkT.T @ memT          (contract d)
  v  [S,e]   = memT.T @ WvT          (contract d)   <- natural lhsT use
  expT[S,Tq] = exp(scale * kT_s.T @ qT)             (contract r, single MM)
  UT [e,Tq]  = v_s.T @ expT          (contract S)
  out [T,o]  = UT_t.T @ WoT          (contract e)
Softmax: logits here are bounded (|x| < ~10), so exp is fp32-safe with
no max subtraction; row-sums come from a ones-vector matmul and the
division is folded into the final PSUM->SBUF scaling (per-partition
scalar multiply). All matmuls run as float32r (full PE rate at moving
free-dim >= 256, reduced mantissa) on fp32 data.
"""

import ml_dtypes
import numpy as np

import concourse.bass as bass
import concourse.mybir as mybir
import concourse.tile as tile
from concourse.bacc import Bacc
from concourse.bass_utils import run_bass_kernel_spmd

FP = mybir.dt.float32
FR = mybir.dt.float32r
BF = mybir.dt.bfloat16
ts = bass.ts

B = 4
T_FULL = 2048
D = 1024
R = 128
S = 2048
E = 1024
O = 1024
T = 1024            # per-core query tokens (half of T_FULL)
P = 128
SCALE = 1.0 / np.sqrt(128.0)

KD = D // P         # 8 contraction tiles over d
NS = S // P         # 16 key/value tiles
NE = E // P         # 8 value-feature tiles
TQ = 256            # query-column strip processed per attention pass
NQ = T // TQ        # 4 strips

# Set by test harness to enable NTFF profiling; LAST_RESULT holds the
# BassKernelResults of the most recent kernel() call.
TRACE = False
LAST_RESULT = None
_PROG = None


def _build_program(linearize=False):
    # Bacc (not raw Bass): its finalize() runs move_matmul_waits_to_ldweights
    # + generate_event_semaphores, which split multi-sem waits down to the
    # one-wait-per-instruction limit of the TRN2 ISA. Raw Bass trips
    # walrus's "Too many sync wait commands" codegen error.
    nc = Bacc()

    tgtT_d = nc.dram_tensor("tgtT", [D, T], BF, kind="ExternalInput")
    memT_d = nc.dram_tensor("memT", [D, S], BF, kind="ExternalInput")
    wq_d = nc.dram_tensor("WqT", [D, R], BF, kind="ExternalInput")
    wk_d = nc.dram_tensor("WkT", [D, R], BF, kind="ExternalInput")
    wv_d = nc.dram_tensor("WvT", [D, E], BF, kind="ExternalInput")
    wo_d = nc.dram_tensor("WoT", [E, O], BF, kind="ExternalInput")
    out_d = nc.dram_tensor("out", [T, O], FP, kind="ExternalOutput")

    Exp = mybir.ActivationFunctionType.Exp

    with tile.TileContext(nc, linearize=linearize) as tc:
        with tc.tile_pool(name="perm", bufs=1) as perm, \
             tc.tile_pool(name="dram", bufs=1, space="DRAM") as dpool, \
             tc.tile_pool(name="expp", bufs=4) as expp, \
             tc.tile_pool(name="utsb", bufs=2) as utsb, \
             tc.tile_pool(name="outp", bufs=3) as outp, \
             tc.tile_pool(name="rcp", bufs=4) as rcp:
            qT = perm.tile([P, T], BF, tag="qT")
            kT = perm.tile([P, S], BF, tag="kT")
            v = [perm.tile([P, E], BF, tag=f"v{m}", name=f"v{m}") for m in range(NS)]
            ones = perm.tile([P, 1], BF, tag="ones")
            nc.vector.memset(ones, 1.0)
            recip_d = dpool.tile([1, T], FP)

            memT = [perm.tile([P, S], BF, tag=f"m{k}", name=f"m{k}") for k in range(KD)]
            wk = [perm.tile([P, R], BF, tag=f"wk{k}", name=f"wk{k}") for k in range(KD)]
            tgt = [perm.tile([P, T], BF, tag=f"t{k}", name=f"t{k}") for k in range(KD)]
            wq = [perm.tile([P, R], BF, tag=f"wq{k}", name=f"wq{k}") for k in range(KD)]
            wv = [perm.tile([P, E], BF, tag=f"wv{k}", name=f"wv{k}") for k in range(KD)]
            wo = [perm.tile([P, O], BF, tag=f"wo{k}", name=f"wo{k}") for k in range(NE)]
            for k in range(KD):
                nc.sync.dma_start(out=tgt[k], in_=tgtT_d[ts(k, P), :])
                nc.sync.dma_start(out=wq[k], in_=wq_d[ts(k, P), :])
                nc.sync.dma_start(out=memT[k], in_=memT_d[ts(k, P), :])
                nc.sync.dma_start(out=wk[k], in_=wk_d[ts(k, P), :])
                nc.sync.dma_start(out=wv[k], in_=wv_d[ts(k, P), :])
                nc.sync.dma_start(out=wo[k], in_=wo_d[ts(k, P), :])

            # ---- Phase A: projections (qT, kT, v) ----
            with tc.tile_pool(name="psA", bufs=4, space="PSUM") as psA:
                for n in range(T // 512):
                    ps = psA.tile([P, 512], FP)
                    for k in range(KD):
                        nc.tensor.matmul(ps, wq[k],
                                         tgt[k][:, ts(n, 512)],
                                         start=(k == 0), stop=(k == KD - 1))
                    nc.vector.tensor_copy(qT[:, ts(n, 512)], ps)

                for n in range(S // 512):
                    ps = psA.tile([P, 512], FP)
                    for k in range(KD):
                        nc.tensor.matmul(ps, wk[k],
                                         memT[k][:, ts(n, 512)],
                                         start=(k == 0), stop=(k == KD - 1))
                    nc.vector.tensor_copy(kT[:, ts(n, 512)], ps)

                for m in range(NS):
                    for eh in range(E // 512):
                        ps = psA.tile([P, 512], FP)
                        for k in range(KD):
                            nc.tensor.matmul(ps, memT[k][:, ts(m, P)],
                                             wv[k][:, ts(eh, 512)],
                                             start=(k == 0), stop=(k == KD - 1))
                        nc.vector.tensor_copy(v[m][:, ts(eh, 512)], ps)

            # ---- Phase B: attention + output projection, per 256-col strip ----
            with tc.tile_pool(name="psc", bufs=2, space="PSUM") as psc, \
                 tc.tile_pool(name="psums", bufs=1, space="PSUM") as psums, \
                 tc.tile_pool(name="psut", bufs=1, space="PSUM") as psut, \
                 tc.tile_pool(name="pso", bufs=1, space="PSUM") as pso:
                for q in range(NQ):
                    tq = slice(q * TQ, (q + 1) * TQ)
                    sums_ps = psums.tile([1, TQ], FP)
                    ut_ps = [psut.tile([P, 2 * TQ], FP, tag=f"ut{j}", name=f"ut{j}")
                             for j in range(NE // 2)]

                    def scores_exp(s, tq=tq):
                        sc = psc.tile([P, TQ], FP)
                        nc.tensor.matmul(sc, kT[:, ts(s, P)],
                                         qT[:, tq], start=True, stop=True)
                        ex = expp.tile([P, TQ], BF)
                        nc.scalar.activation(ex, sc, Exp, scale=float(SCALE))
                        return ex

                    ex_cur = scores_exp(0)
                    for s in range(NS):
                        ex_next = scores_exp(s + 1) if s + 1 < NS else None
                        first, last = (s == 0), (s == NS - 1)
                        nc.tensor.matmul(sums_ps, ones, ex_cur,
                                         start=first, stop=last)
                        for e in range(NE):
                            j, jj = divmod(e, 2)
                            # start=True clears has_written for the WHOLE
                            # PSUM bank; each ut bank holds two accumulation
                            # groups (jj=0,1), so only the first may clear.
                            # The jj=1 group's first matmul lands on cleared
                            # bits and overwrite+sets them (per-element
                            # accumulate semantics), which is exactly the
                            # start behavior it needs.
                            nc.tensor.matmul(ut_ps[j][:, ts(jj, TQ)],
                                             v[s][:, ts(e, P)], ex_cur,
                                             start=(first and jj == 0),
                                             stop=last)
                        ex_cur = ex_next

                    rcs = rcp.tile([1, TQ], FP, tag="rcs")
                    nc.vector.reciprocal(rcs, sums_ps)
                    nc.sync.dma_start(out=recip_d[0:1, tq], in_=rcs)

                    ut_sb = [utsb.tile([P, 2 * TQ], BF, tag=f"us{j}", name=f"us{j}")
                             for j in range(NE // 2)]
                    for j in range(NE // 2):
                        nc.vector.tensor_copy(ut_sb[j], ut_ps[j])

                    for tt in range(TQ // P):
                        tg = q * (TQ // P) + tt
                        rc = rcp.tile([P, 1], FP, tag="rc")
                        nc.sync.dma_start(
                            out=rc,
                            in_=recip_d[0:1, ts(tg, P)].rearrange("a b -> b a"))
                        for oh in range(O // 512):
                            po = pso.tile([P, 512], FP)
                            for e in range(NE):
                                j, jj = divmod(e, 2)
                                lhs = ut_sb[j][:, jj * TQ + tt * P:
                                               jj * TQ + (tt + 1) * P]
                                nc.tensor.matmul(po, lhs,
                                                 wo[e][:, ts(oh, 512)],
                                                 start=(e == 0), stop=(e == NE - 1))
                            ob = outp.tile([P, 512], FP)
                            nc.vector.tensor_scalar_mul(ob, po, rc)
                            nc.sync.dma_start(out=out_d[ts(tg, P), ts(oh, 512)],
                                              in_=ob)
    return nc


def kernel(tgt, memory, Wq, Wk, Wv, Wo):
    """8-way data-parallel (batch x query-half) low-rank cross-attention
    on the 8 NeuronCores via the Bass/Tile kernel above."""
    global LAST_RESULT, _PROG

    tgt = np.asarray(tgt, dtype=np.float32)
    memory = np.asarray(memory, dtype=np.float32)
    BFnp = ml_dtypes.bfloat16

    wqT = np.ascontiguousarray(np.asarray(Wq, np.float32).T).astype(BFnp)
    wkT = np.ascontiguousarray(np.asarray(Wk, np.float32).T).astype(BFnp)
    wvT = np.ascontiguousarray(np.asarray(Wv, np.float32).T).astype(BFnp)
    woT = np.ascontiguousarray(np.asarray(Wo, np.float32).T).astype(BFnp)

    in_maps = []
    for c in range(8):
        b, h = divmod(c, 2)
        tgtT = np.ascontiguousarray(
            tgt[b, h * T:(h + 1) * T, :].T).astype(BFnp)        # [D, T]
        memT = np.ascontiguousarray(memory[b].T).astype(BFnp)   # [D, S]
        in_maps.append({"tgtT": tgtT, "memT": memT,
                        "WqT": wqT, "WkT": wkT, "WvT": wvT, "WoT": woT})

    if _PROG is None:
        _PROG = _build_program()
        # Bacc defers register allocation to finalize(); the bass_exec
        # lowering serializes the module as-is, so finalize here or walrus
        # sees reg_id=-1 ("Reg has not been allocated yet").
        _PROG.finalize()
    res = run_bass_kernel_spmd(_PROG, in_maps, core_ids=list(range(8)),
                               trace=TRACE)
    LAST_RESULT = res

    out = np.empty((B, T_FULL, O), dtype=np.float32)
    for c in range(8):
        b, h = divmod(c, 2)
        out[b, h * T:(h + 1) * T, :] = res.results[c]["out"]
    return out

